# revision 39
# baseline (speedup 1.0000x reference)
"""Multi-head causal self-attention (B=4, N=2048, D=1024, H=16) on 8 TRN2 cores.

Sharding: 8 cores = 4 batches x 2 head-groups (8 heads / 512 dims each).
Per core (batch b, group g):
  - QKV projections computed in transposed layout (dims on partitions):
      Q^T, K^T = W^T-chunks (lhsT) x x^T (rhs), accumulated over 8 din chunks.
      V computed in natural [token, dv] layout (lhsT = x^T chunk).
  - Attention computed as S^T tiles [keys(128) x queries(512)] so that
    exp(S) feeds the P^T.V matmul directly (contraction over keys on
    partitions, no transposes anywhere). The two heads of a 128-dim strip
    occupy SBUF partitions 0:64 / 64:128, so their S^T matmuls (contraction
    64 = head_dim) lower to PE row-tiles (64,0)/(64,128-mode T0/T8) and run
    CONCURRENTLY in the array -- emitted back-to-back into one 2-bank PSUM
    tile, exp'd in a single wide activation covering both heads.
    Softmax denominators come from a ones-column appended to V (row HD of
    the PV accumulator); normalization is deferred and batched per strip.
    Causal masking = skip blocks above the diagonal + multiply
    diagonal-region tiles by a precomputed 0/1 mask after exp. No
    max-subtraction: scores are ~N(0,1) after the 1/sqrt(hd) scale.
  - O-projection partial: attnT (lhsT) x Wo-slice (rhs) -> [2048, 1024]
    partial output per core; host sums the two group partials per batch.
    O-proj for early token tiles is interleaved into the last strip's
    attention (per-query-strip fast normalize) so the PE never idles
    waiting on the scalar engine's exp throughput.

Dtypes: bf16 matmuls everywhere (full PE rate); f32 PSUM accumulation.
"""

import itertools

import numpy as np
import ml_dtypes

import concourse.bass as bass
import concourse.tile as tile
from concourse import bacc, mybir
from concourse import bass_utils
from concourse._compat import with_exitstack
from concourse.bass import ts, ds

B, N, D, H, HD = 4, 2048, 1024, 16, 64
GROUPS = 2              # head groups (cores per batch)
DC = D // GROUPS        # 512 dims per core
HPC = H // GROUPS       # 8 heads per core
P = 128
QW = 512                # query strip width / matmul free dim
NDIN = D // P           # 8 contraction chunks for QKV
NSTRIP = DC // P        # 4 dq strips per core (2 heads each)
NTT = N // P            # 16 token tiles
NTS = N // QW           # 4 token strips
NQB = QW // P           # 4 query blocks per strip

F32 = mybir.dt.float32
BF16 = mybir.dt.bfloat16


def _emit(ctx, tc, xT, wq, wk, wv, wo, bq, bk, bv, masks, out):
    nc = tc.nc
    EXP = mybir.ActivationFunctionType.Exp

    const = ctx.enter_context(tc.tile_pool(name="const", bufs=1))
    p_pt = ctx.enter_context(tc.tile_pool(name="p_pt", bufs=4))
    p_small = ctx.enter_context(tc.tile_pool(name="p_small", bufs=2))
    p_dram = ctx.enter_context(tc.tile_pool(name="p_dram", bufs=2, space="DRAM"))
    # PSUM budget (8 banks): p_st 2x2 + p_pv 1x2 + p_aux 2x1 = 8.
    p_st = ctx.enter_context(tc.tile_pool(name="p_st", bufs=2, space="PSUM"))
    p_pv = ctx.enter_context(tc.tile_pool(name="p_pv", bufs=1, space="PSUM"))
    p_aux = ctx.enter_context(tc.tile_pool(name="p_aux", bufs=2, space="PSUM"))
    p_osb = ctx.enter_context(tc.tile_pool(name="p_osb", bufs=3))

    # constants on the GpSimd (SWDGE) queue so they don't serialize with the
    # x^T stream on the sync (HWDGE) queue. maskt = one triangular 0/1 tile.
    maskt = const.tile([P, P], BF16)
    nc.gpsimd.dma_start(out=maskt, in_=masks)
    bqt = const.tile([P, NSTRIP], F32)
    nc.gpsimd.dma_start(out=bqt, in_=bq.rearrange("(s p) -> p s", p=P))
    bkt = const.tile([P, NSTRIP], F32)
    nc.gpsimd.dma_start(out=bkt, in_=bk.rearrange("(s p) -> p s", p=P))
    bvb = const.tile([P, DC], F32)      # loaded after strip-0 criticals

    # persistent per-batch tensors
    attnT = const.tile([P, NSTRIP, N], BF16)                # normalized attn^T
    vplus = const.tile([P, NTT, HPC, HD + 1], BF16)         # V | ones column
    # memset on a bf16 matmul-input tile is invalid ISA; write the ones
    # column via a DVE copy from an f32 staging tile (a valid rounding producer)
    ones_f32 = const.tile([P, NTT * HPC], F32)
    nc.vector.memset(ones_f32, 1.0)
    nc.vector.tensor_copy(
        out=vplus[:, :, :, HD:HD + 1],
        in_=ones_f32.rearrange("p (a b) -> p a b", b=HPC).unsqueeze(3),
    )
    # selectors for the PE-broadcast normalize: [128,128] stationaries with
    # one all-ones row (32*qs) and zeros elsewhere, so lhsT.T @ recip_rows
    # replicates that reciprocal row across all 128 PSUM partitions. The
    # reciprocal input stays full-width at partition 0 (custom-DVE ops
    # misbehave on partition-offset slices); unselected moving rows are
    # finite (sums tiles are memset 1.0), so the 0-weights stay exact.
    sel_f32 = const.tile([P, P], F32)
    nc.vector.memset(sel_f32, 0.0)
    sel_bf = []
    for i in range(NTS):
        sb = const.tile([P, P], BF16, name=f"sel{i}")
        nc.vector.tensor_copy(out=sb, in_=sel_f32)
        nc.vector.tensor_copy(
            out=sb[32 * i:32 * i + 1, :],
            in_=ones_f32[0:1, 0:P])
        sel_bf.append(sb)

    # Wo tile loaded after strip-0 weights (not needed until phase C); single
    # multi-chunk descriptor per tensor -- each dma_start costs ~600ns of issue
    # time on its queue engine, so batching chunks into one AP matters. All
    # weight/x DRAM tensors arrive host-pre-tiled so every DMA reads 8KB
    # contiguous per partition (1KB segments halve effective DMA bandwidth).
    wot = const.tile([P, NSTRIP, D], BF16)

    with tc.tile_pool(name="p_xt", bufs=1) as p_xt:
        # x^T resident, 64KB/part total, one tile per strip so dependency
        # tracking can't couple strip-0 consumers to strip 1-3 DMAs.
        # Strip 0 is split into three tiles (c0 | c1-3 | c4-7) so the first
        # QK matmul starts after only one weight chunk + one x chunk land.
        xts = [p_xt.tile([P, NDIN // 2, QW], BF16, name="xts0a"),
               p_xt.tile([P, NDIN // 2, QW], BF16, name="xts0b")]
        xts += [p_xt.tile([P, NDIN, QW], BF16, name=f"xts{t}")
                for t in range(1, NTS)]

        def xtile(t, c):
            if t == 0:
                return xts[c // 4][:, c % 4, :]
            return xts[t + 1][:, c, :]

        with (
            tc.tile_pool(name="p_w", bufs=2) as p_w,
            tc.tile_pool(name="p_wv", bufs=1) as p_wv,
            tc.tile_pool(name="p_qk", bufs=2) as p_qk,
        ):
            # the sync queue moves ~4x the bytes/ns of the scalar/gpsimd
            # queues, so the whole critical sequence rides it in exact
            # consumption order. Only deferrable loads (bv, strip>=1
            # weights, Wo) go elsewhere.
            wqs0 = p_w.tile([P, NDIN, P], BF16, tag="wq")
            wks0 = p_w.tile([P, NDIN, P], BF16, tag="wk")
            nc.sync.dma_start(out=wqs0, in_=wq[0])
            nc.sync.dma_start(out=xts[0], in_=xT[:, 0, 0:4])
            nc.sync.dma_start(out=wks0, in_=wk[0])
            nc.sync.dma_start(out=xts[1], in_=xT[:, 0, 4:8])
            nc.sync.dma_start(out=xts[2], in_=xT[:, 1])
            wvt = p_wv.tile([P, NDIN, DC], BF16)
            nc.sync.dma_start(out=wvt, in_=wv)
            nc.sync.dma_start(out=xts[3], in_=xT[:, 2])
            nc.sync.dma_start(out=xts[4], in_=xT[:, 3])
            nc.gpsimd.dma_start(
                out=bvb, in_=bv.unsqueeze(0).partition_broadcast(P))

            st = {}
            pending = [None]

            def make_strip(s):
                if s == 0:
                    def wq_at(c):
                        return wqs0[:, c, :]

                    def wk_at(c):
                        return wks0[:, c, :]
                else:
                    wqs = p_w.tile([P, NDIN, P], BF16, tag="wq")
                    wks = p_w.tile([P, NDIN, P], BF16, tag="wk")
                    nc.gpsimd.dma_start(out=wqs, in_=wq[s])
                    nc.gpsimd.dma_start(out=wks, in_=wk[s])

                    def wq_at(c, w=wqs):
                        return w[:, c, :]

                    def wk_at(c, w=wks):
                        return w[:, c, :]
                if s == 1:
                    nc.gpsimd.dma_start(out=wot, in_=wo)
                qts = p_qk.tile([P, N], BF16, tag="qt")
                kts = p_qk.tile([P, N], BF16, tag="kt")
                # sums rows at partition offsets {0,32,64,96} (DVE partition
                # offsets must be 32-aligned); unused rows are memset to 1.0
                # so the batched reciprocal stays finite
                sums_sb = p_small.tile([P, 2, QW], F32, tag="sums")
                nc.gpsimd.memset(sums_sb, 1.0)
                st[s] = (wq_at, wk_at, qts, kts, sums_sb)

            def s_units(qs):
                """Unit list for one query strip: full-width key blocks
                below the diagonal region, then three diagonal units with
                shrinking query widths -- queries before the key block are
                skipped entirely, the remaining 128-wide leading wedge of
                each unit gets the triangular mask."""
                nfull = NQB * qs
                us = [("full", kb) for kb in range(nfull)]
                us += [("diag0", nfull), ("diag1", nfull), ("diagB", nfull)]
                return us

            def emit_s(s, qs, kind, kb):
                """S^T + exp for both heads of one unit. The two heads'
                S matmuls (contraction 64, partition bases 0/64) occupy
                disjoint PE row-tiles and run concurrently; one activation
                exps both heads' scores."""
                _, _, qts, kts, _ = st[s]
                q0 = qs * QW
                pst = p_st.tile([P, 2, QW], F32, tag="st", name="pst")
                pt = p_pt.tile([P, 2, QW], BF16, tag="pt", name="pt")
                if kind == "full" or kind == "diag0":
                    for h2 in range(2):
                        po = h2 * HD
                        nc.tensor.matmul(
                            pst[:, h2, :],
                            lhsT=kts[po:po + HD, ts(kb, P)],
                            rhs=qts[po:po + HD, ts(qs, QW)],
                            start=True, stop=True,
                        )
                    nc.scalar.activation(out=pt, in_=pst, func=EXP,
                                         scale=0.125)
                    if kind == "diag0":
                        for h2 in range(2):
                            nc.vector.tensor_mul(
                                pt[:, h2, 0:P], pt[:, h2, 0:P], maskt)
                    return pt
                if kind == "diag1":
                    # kc = kb+1, queries [128:512), tri on cols 0:128
                    for h2 in range(2):
                        po = h2 * HD
                        nc.tensor.matmul(
                            pst[:, h2, 0:3 * P],
                            lhsT=kts[po:po + HD, ts(kb + 1, P)],
                            rhs=qts[po:po + HD, ds(q0 + P, 3 * P)],
                            start=True, stop=True,
                        )
                    nc.scalar.activation(
                        out=pt[:, :, 0:3 * P], in_=pst[:, :, 0:3 * P],
                        func=EXP, scale=0.125)
                    for h2 in range(2):
                        nc.vector.tensor_mul(
                            pt[:, h2, 0:P], pt[:, h2, 0:P], maskt)
                    return pt
                # diagB: kc=kb+2, queries [256:512) at cols 0:256;
                #        kc=kb+3, queries [384:512) at cols 256:384
                for h2 in range(2):
                    po = h2 * HD
                    nc.tensor.matmul(
                        pst[:, h2, 0:2 * P],
                        lhsT=kts[po:po + HD, ts(kb + 2, P)],
                        rhs=qts[po:po + HD, ds(q0 + 2 * P, 2 * P)],
                        start=True, stop=True,
                    )
                for h2 in range(2):
                    po = h2 * HD
                    nc.tensor.matmul(
                        pst[:, h2, 2 * P:3 * P],
                        lhsT=kts[po:po + HD, ts(kb + 3, P)],
                        rhs=qts[po:po + HD, ds(q0 + 3 * P, P)],
                        start=True, stop=True,
                    )
                nc.scalar.activation(
                    out=pt[:, :, 0:3 * P], in_=pst[:, :, 0:3 * P],
                    func=EXP, scale=0.125)
                for h2 in range(2):
                    nc.vector.tensor_mul(
                        pt[:, h2, 0:P], pt[:, h2, 0:P], maskt)
                    nc.vector.tensor_mul(
                        pt[:, h2, 2 * P:3 * P], pt[:, h2, 2 * P:3 * P],
                        maskt)
                return pt

            def emit_pv(s, kind, kb, pt, pvp):
                for h2 in range(2):
                    h = 2 * s + h2
                    if kind == "full" or kind == "diag0":
                        nc.tensor.matmul(
                            pvp[:, h2, :], lhsT=vplus[:, kb, h, :],
                            rhs=pt[:, h2, :],
                            start=(kb == 0), stop=False,
                        )
                    elif kind == "diag1":
                        nc.tensor.matmul(
                            pvp[:, h2, P:4 * P],
                            lhsT=vplus[:, kb + 1, h, :],
                            rhs=pt[:, h2, 0:3 * P],
                            start=False, stop=False,
                        )
                    else:
                        nc.tensor.matmul(
                            pvp[:, h2, 2 * P:4 * P],
                            lhsT=vplus[:, kb + 2, h, :],
                            rhs=pt[:, h2, 0:2 * P],
                            start=False, stop=False,
                        )
                        nc.tensor.matmul(
                            pvp[:, h2, 3 * P:4 * P],
                            lhsT=vplus[:, kb + 3, h, :],
                            rhs=pt[:, h2, 2 * P:3 * P],
                            start=False, stop=True,
                        )

            def evict_qs(s, qs, pvp):
                # evict both heads in parallel (scalar takes one attnT copy,
                # vector the other): pvp is single-buffered, so these gate
                # the next query strip's first PV matmuls
                _, _, _, _, sums_sb = st[s]
                for h2 in range(2):
                    nc.vector.tensor_copy(
                        out=sums_sb[32 * qs:32 * qs + 1, h2, :],
                        in_=pvp[HD:HD + 1, h2, :])
                nc.scalar.copy(
                    out=attnT[0:HD, s, ts(qs, QW)], in_=pvp[0:HD, 0, :])
                nc.vector.tensor_copy(
                    out=attnT[HD:2 * HD, s, ts(qs, QW)], in_=pvp[0:HD, 1, :])

            def flat_attention(items, filler=None, drain_hook=None):
                """LOOKP-pipelined S/exp stream with PV trailing, flat
                across query-strip and head-strip boundaries so the PE
                never drains between them. items: (s, qs, kind, kb, last).
                drain_hook(s, qs) runs after each strip's eviction."""
                LOOKP = 2
                pts = {}
                pvps = {}
                n = len(items)
                for i in range(n + LOOKP):
                    if i < n:
                        s, qs, kind, kb, _ = items[i]
                        pts[i] = emit_s(s, qs, kind, kb)
                    if filler is not None:
                        next(filler, None)
                    if i >= LOOKP:
                        s, qs, kind, kb, last = items[i - LOOKP]
                        if (s, qs) not in pvps:
                            pvps[(s, qs)] = p_pv.tile(
                                [HD + 1, 2, QW], F32, tag="pv", name="pvp")
                        emit_pv(s, kind, kb, pts.pop(i - LOOKP),
                                pvps[(s, qs)])
                        if last:
                            evict_qs(s, qs, pvps.pop((s, qs)))
                            if drain_hook is not None:
                                drain_hook(s, qs)

            def attn_pair(s, qs, filler=None):
                us = s_units(qs)
                items = [(s, qs, kind, kb, i == len(us) - 1)
                         for i, (kind, kb) in enumerate(us)]
                flat_attention(items, filler)

            def normalize_h2(s, h2, sums_sb):
                """Batched softmax normalization for one head (4 query strips).

                1/s via the fast custom-DVE reciprocal; normalize multiplies
                run on GpSimd to keep Vector free for the inner-loop copies.
                """
                po = h2 * HD
                recip_sb = p_small.tile([P, QW], F32, tag="recip",
                                        name="recip_sb")
                nc.vector.reciprocal_approx_fast(
                    out=recip_sb, in_=sums_sb[:, h2, :])
                recb_sb = p_small.tile([P, QW], BF16, tag="recb",
                                       name="recb_sb")
                nc.vector.tensor_copy(out=recb_sb, in_=recip_sb)
                # broadcast across partitions via a DRAM round-trip
                # (SBUF-source partition-broadcast DMA is rejected); bf16
                # halves the 1MB/head broadcast traffic
                recip_d = p_dram.tile([NTS, QW], BF16, tag="recipd",
                                      name="recip_d")
                nc.sync.dma_start(
                    out=recip_d,
                    in_=recb_sb.rearrange("(a b) f -> a b f", b=32)[:, 0, :])
                # full-128-partition broadcast so rb[po:po+HD] shares the
                # base partition with the attnT slice (DVE rule); all 4
                # query strips in one issue
                rb = p_small.tile([P, NTS, QW], BF16, tag="rb", bufs=2,
                                  name="rb")
                nc.sync.dma_start(
                    out=rb, in_=recip_d.unsqueeze(0).partition_broadcast(P))
                for qs in range(NTS):
                    sl = attnT[po:po + HD, s, ts(qs, QW)]
                    nc.gpsimd.tensor_mul(
                        out=sl, in0=sl, in1=rb[po:po + HD, qs, :])

            def normalize_bcast_qs(s, h2, qs, sums_sb):
                """Per-query-strip normalize for the LAST strip: reciprocal
                of the sums rows, with the wanted row replicated across
                partitions by a one-hot PE matmul (sel[128,128] x
                recip[128,512] -> PSUM[128,512]), then one DVE multiply.
                ~2us end-to-end vs ~6us for the DRAM round-trip broadcast,
                so the interleaved O-projection isn't gated on a long
                store/load chain."""
                po = h2 * HD
                recq = p_small.tile([P, QW], F32, tag="recq", name="recq")
                nc.vector.reciprocal_approx_fast(
                    out=recq, in_=sums_sb[:, h2, :])
                recb = p_small.tile([P, QW], BF16, tag="recb2", name="recb2")
                # the bf16 cast rides scalar for one head so the two heads'
                # chains run on different engines
                if h2:
                    nc.scalar.copy(out=recb, in_=recq)
                else:
                    nc.vector.tensor_copy(out=recb, in_=recq)
                rbq = p_aux.tile([P, QW], F32, tag="mm", name="rbq")
                nc.tensor.matmul(
                    rbq, lhsT=sel_bf[qs], rhs=recb,
                    start=True, stop=True)
                sl = attnT[po:po + HD, s, ts(qs, QW)]
                nc.vector.tensor_mul(
                    out=sl, in0=sl, in1=rbq[po:po + HD, :])

            def emit_qk0(t):
                """Strip-0 Q/K projection for one token strip (plain order:
                runs against the incoming x/weight DMA stream)."""
                wq_at, wk_at, qts, kts, _ = st[0]
                psq = p_aux.tile([P, QW], F32, tag="mm", name="psq")
                for c in range(NDIN):
                    nc.tensor.matmul(
                        psq, lhsT=wq_at(c), rhs=xtile(t, c),
                        start=(c == 0), stop=(c == NDIN - 1),
                    )
                nc.vector.tensor_scalar_add(
                    out=qts[:, ts(t, QW)], in0=psq, scalar1=bqt[:, 0:1])
                psk = p_aux.tile([P, QW], F32, tag="mm", name="psk")
                for c in range(NDIN):
                    nc.tensor.matmul(
                        psk, lhsT=wk_at(c), rhs=xtile(t, c),
                        start=(c == 0), stop=(c == NDIN - 1),
                    )
                nc.vector.tensor_scalar_add(
                    out=kts[:, ts(t, QW)], in0=psk, scalar1=bkt[:, 0:1])

            def emit_v(t):
                # V = x @ Wv + bv, one token strip at a time right
                # before the attention group that first needs it
                for tt in range(NQB * t, NQB * (t + 1)):
                    psv = p_aux.tile([P, DC], F32, tag="mm", name="psv")
                    for c in range(NDIN):
                        nc.tensor.matmul(
                            psv,
                            lhsT=xtile(t, c)[:, ts(tt % NQB, P)],
                            rhs=wvt[:, c, :],
                            start=(c == 0), stop=(c == NDIN - 1),
                        )
                    nc.vector.tensor_add(
                        out=vplus[:, tt, :, 0:HD],
                        in0=psv.rearrange("p (h d) -> p h d", d=HD),
                        in1=bvb.rearrange("p (h d) -> p h d", d=HD),
                    )

            def qk_pair_gen(s, t0s):
                """Chunk-major Q/K projection for strip s over the given
                token-strip pairs: each weight chunk is loaded once as the
                PE stationary and used for two token strips' matmuls.
                Yields after every chunk (~2 matmuls) so the attention loop
                can pull fine-grained PE filler."""
                wq_at, wk_at, qts, kts, _ = st[s]
                for w_at, dst, bias in ((wq_at, qts, bqt), (wk_at, kts, bkt)):
                    for t0 in t0s:
                        ps0 = p_aux.tile([P, QW], F32, tag="mm", name="ps0")
                        ps1 = p_aux.tile([P, QW], F32, tag="mm", name="ps1")
                        for c in range(NDIN):
                            nc.tensor.matmul(
                                ps0, lhsT=w_at(c), rhs=xtile(t0, c),
                                start=(c == 0), stop=(c == NDIN - 1))
                            nc.tensor.matmul(
                                ps1, lhsT=w_at(c), rhs=xtile(t0 + 1, c),
                                start=(c == 0), stop=(c == NDIN - 1))
                            yield
                        nc.vector.tensor_scalar_add(
                            out=dst[:, ts(t0, QW)], in0=ps0,
                            scalar1=bias[:, s:s + 1])
                        nc.vector.tensor_scalar_add(
                            out=dst[:, ts(t0 + 1, QW)], in0=ps1,
                            scalar1=bias[:, s:s + 1])
                        yield

            def qk_part1(s):
                """Strip s's setup + Q/K for tokens 0:1024 -- pulled as PE
                filler during strip s-1's attention. Tokens 1024:2048
                (qk_part2) are deferred into strip s's own first two query
                strips, which otherwise have no filler work."""
                make_strip(s)
                yield
                yield from qk_pair_gen(s, (0,))

            def qk_part2(s):
                yield from qk_pair_gen(s, (2,))

            def phase_c_tts(tts):
                """O-projection for the given token tiles: partial output =
                attnT^T @ Wo_slice. The stationary attnT chunk is shared by
                the two output-half matmuls."""
                for tt in tts:
                    pso0 = p_aux.tile([P, QW], F32, tag="mm", name="pso0")
                    pso1 = p_aux.tile([P, QW], F32, tag="mm", name="pso1")
                    for c in range(NSTRIP):
                        nc.tensor.matmul(
                            pso0, lhsT=attnT[:, c, ts(tt, P)],
                            rhs=wot[:, c, ds(0, QW)],
                            start=(c == 0), stop=(c == NSTRIP - 1))
                        nc.tensor.matmul(
                            pso1, lhsT=attnT[:, c, ts(tt, P)],
                            rhs=wot[:, c, ds(QW, QW)],
                            start=(c == 0), stop=(c == NSTRIP - 1))
                    # bf16 out + store each half immediately: halves the
                    # store bytes and drains right after the copy
                    osb = p_osb.tile([P, D], BF16, tag="osb", name="osb")
                    nc.vector.tensor_copy(out=osb[:, ds(0, QW)], in_=pso0)
                    nc.sync.dma_start(
                        out=out[ts(tt, P), ds(0, QW)], in_=osb[:, ds(0, QW)])
                    nc.vector.tensor_copy(out=osb[:, ds(QW, QW)], in_=pso1)
                    nc.sync.dma_start(
                        out=out[ts(tt, P), ds(QW, QW)],
                        in_=osb[:, ds(QW, QW)])

            # ---- strip 0: software-pipelined against the DMA stream ----
            make_strip(0)
            _, _, _, _, sums0 = st[0]
            emit_qk0(0)
            qk1 = None
            for t in range(NTS):
                if t + 1 < NTS:
                    emit_qk0(t + 1)
                emit_v(t)
                attn_pair(0, t)
                if t == 0:
                    qk1 = qk_part1(1)
                for _ in range(9):
                    next(qk1, None)
            for _ in qk1:
                pass
            pending[0] = (lambda sb=sums0:
                          (normalize_h2(0, 0, sb),
                           normalize_h2(0, 1, sb)))

            # ---- strips 1-3 ----
            for s in range(1, NSTRIP):
                _, _, _, _, sums_sb = st[s]
                last = (s == NSTRIP - 1)
                filler = itertools.chain(
                    qk_part2(s),
                    qk_part1(s + 1) if not last else iter(()))
                for qs in range(NTS):
                    if last and qs >= 1:
                        # one token tile held back from the previous group
                        # goes FIRST: it is ready-to-run PE work covering
                        # this normalize chain's latency (critical for the
                        # final one, where no attention work remains)
                        if qs >= 2:
                            phase_c_tts([NQB * (qs - 2) + 3])
                        normalize_bcast_qs(s, 0, qs - 1, sums_sb)
                        normalize_bcast_qs(s, 1, qs - 1, sums_sb)
                        phase_c_tts(range(NQB * (qs - 1), NQB * qs - 1))
                    attn_pair(s, qs, filler)
                    if pending[0] is not None and qs == (0 if last else 1):
                        pending[0]()
                        pending[0] = None
                if not last:
                    for _ in filler:
                        pass
                    pending[0] = (lambda ss=s, sb=sums_sb:
                                  (normalize_h2(ss, 0, sb),
                                   normalize_h2(ss, 1, sb)))
                else:
                    phase_c_tts([NQB * (NTS - 2) + 3])
                    normalize_bcast_qs(s, 0, NTS - 1, sums_sb)
                    normalize_bcast_qs(s, 1, NTS - 1, sums_sb)
                    phase_c_tts(range(NQB * (NTS - 1), NTT))


_emit_wrapped = with_exitstack(_emit)

_NC_CACHE = None


def _build():
    global _NC_CACHE
    if _NC_CACHE is not None:
        return _NC_CACHE
    nc = bacc.Bacc("TRN2", target_bir_lowering=False, debug=False)
    # all inputs host-pre-tiled to the SBUF tile layouts (contiguous
    # per-partition runs -> minimal DMA descriptors)
    xT = nc.dram_tensor(
        "xt", [P, NTS, NDIN, QW], BF16, kind="ExternalInput").ap()
    wq = nc.dram_tensor(
        "wq", [NSTRIP, P, NDIN, P], BF16, kind="ExternalInput").ap()
    wk = nc.dram_tensor(
        "wk", [NSTRIP, P, NDIN, P], BF16, kind="ExternalInput").ap()
    wv = nc.dram_tensor(
        "wv", [P, NDIN, DC], BF16, kind="ExternalInput").ap()
    wo = nc.dram_tensor(
        "wo", [P, NSTRIP, D], BF16, kind="ExternalInput").ap()
    bq = nc.dram_tensor("bq", [DC], F32, kind="ExternalInput").ap()
    bk = nc.dram_tensor("bk", [DC], F32, kind="ExternalInput").ap()
    bv = nc.dram_tensor("bv", [DC], F32, kind="ExternalInput").ap()
    masks = nc.dram_tensor("masks", [P, P], BF16, kind="ExternalInput").ap()
    out = nc.dram_tensor("out", [N, D], BF16, kind="ExternalOutput").ap()
    with tile.TileContext(nc) as tc:
        _emit_wrapped(tc, xT, wq, wk, wv, wo, bq, bk, bv, masks, out)
    nc.compile()
    _NC_CACHE = nc
    return nc


def _make_masks():
    # triangular 0/1 tile for the diagonal blocks of S^T: key <= query kept
    return np.triu(np.ones((P, P), np.float32)).astype(ml_dtypes.bfloat16)


def _in_maps(x, Wq, bq, Wk, bk, Wv, bv, Wo):
    masks = _make_masks()
    bf = ml_dtypes.bfloat16
    maps = []
    # pre-tile into the exact SBUF layouts (see _build dram shapes)
    wq_g, wk_g, wv_g, wo_g = [], [], [], []
    for g in range(GROUPS):
        sl = slice(g * DC, (g + 1) * DC)
        wq_g.append(np.ascontiguousarray(
            Wq[:, sl].reshape(NDIN, P, NSTRIP, P).transpose(2, 1, 0, 3)
        ).astype(bf))
        wk_g.append(np.ascontiguousarray(
            Wk[:, sl].reshape(NDIN, P, NSTRIP, P).transpose(2, 1, 0, 3)
        ).astype(bf))
        wv_g.append(np.ascontiguousarray(
            Wv[:, sl].reshape(NDIN, P, DC).transpose(1, 0, 2)).astype(bf))
        wo_g.append(np.ascontiguousarray(
            Wo[sl, :].reshape(NSTRIP, P, D).transpose(1, 0, 2)).astype(bf))
    for b in range(B):
        xt_b = np.ascontiguousarray(
            np.asarray(x[b]).T.reshape(NDIN, P, NTS, QW).transpose(1, 2, 0, 3)
        ).astype(bf)
        for g in range(GROUPS):
            sl = slice(g * DC, (g + 1) * DC)
            maps.append({
                "xt": xt_b,
                "wq": wq_g[g],
                "wk": wk_g[g],
                "wv": wv_g[g],
                "wo": wo_g[g],
                "bq": np.ascontiguousarray(bq[sl]),
                "bk": np.ascontiguousarray(bk[sl]),
                "bv": np.ascontiguousarray(bv[sl]),
                "masks": masks,
            })
    return maps


def run(inputs, trace=False, tmpdir=None):
    """Build+run on 8 cores. Returns (out [B,N,D] f32, BassKernelResults)."""
    x = np.asarray(inputs["x"], np.float32)
    args = [np.asarray(inputs[k], np.float32) for k in
            ("Wq", "bq", "Wk", "bk", "Wv", "bv", "Wo")]
    bo = np.asarray(inputs["bo"], np.float32)
    nc = _build()
    maps = _in_maps(x, *args)
    if trace:
        bass_utils.upload_artifacts = lambda d: d
    res = bass_utils.run_bass_kernel_spmd(
        nc, maps, core_ids=list(range(8)), trace=trace, tmpdir=tmpdir)
    out = np.empty((B, N, D), np.float32)
    for b in range(B):
        out[b] = (res.results[2 * b]["out"].astype(np.float32)
                  + res.results[2 * b + 1]["out"].astype(np.float32) + bo)
    return out, res


def kernel(**inputs):
    out, _ = run(inputs)
    return out


# revision 42
# speedup vs baseline: 1.1782x; 1.1782x over previous
"""Multi-head causal self-attention (B=4, N=2048, D=1024, H=16) on 8 TRN2 cores.

Sharding: 8 cores = 4 batches x 2 head-groups (8 heads / 512 dims each).
Per core (batch b, group g):
  - QKV projections computed in transposed layout (dims on partitions):
      Q^T, K^T = W^T-chunks (lhsT) x x^T (rhs), accumulated over 8 din chunks.
      V computed in natural [token, dv] layout (lhsT = x^T chunk).
  - Attention computed as S^T tiles [keys(128) x queries(512)] so that
    exp(S) feeds the P^T.V matmul directly (contraction over keys on
    partitions, no transposes anywhere). The two heads of a 128-dim strip
    occupy SBUF partitions 0:64 / 64:128, so their S^T matmuls (contraction
    64 = head_dim) lower to PE row-tiles (64,0)/(64,128-mode T0/T8) and run
    CONCURRENTLY in the array -- emitted back-to-back into one 2-bank PSUM
    tile, exp'd in a single wide activation covering both heads.
    Softmax denominators come from a ones-column appended to V (row HD of
    the PV accumulator); normalization is deferred and batched per strip.
    Causal masking = skip blocks above the diagonal + multiply
    diagonal-region tiles by a precomputed 0/1 mask after exp. No
    max-subtraction: scores are ~N(0,1) after the 1/sqrt(hd) scale.
  - O-projection partial: attnT (lhsT) x Wo-slice (rhs) -> [2048, 1024]
    partial output per core; host sums the two group partials per batch.
    O-proj for early token tiles is interleaved into the last strip's
    attention (per-query-strip fast normalize) so the PE never idles
    waiting on the scalar engine's exp throughput.

Dtypes: bf16 matmuls everywhere (full PE rate); f32 PSUM accumulation.
"""

import itertools

import numpy as np
import ml_dtypes

import concourse.bass as bass
import concourse.tile as tile
from concourse import bacc, mybir
from concourse import bass_utils
from concourse._compat import with_exitstack
from concourse.bass import ts, ds

B, N, D, H, HD = 4, 2048, 1024, 16, 64
GROUPS = 2              # head groups (cores per batch)
DC = D // GROUPS        # 512 dims per core
HPC = H // GROUPS       # 8 heads per core
P = 128
QW = 512                # query strip width / matmul free dim
NDIN = D // P           # 8 contraction chunks for QKV
NSTRIP = DC // P        # 4 dq strips per core (2 heads each)
NTT = N // P            # 16 token tiles
NTS = N // QW           # 4 token strips
NQB = QW // P           # 4 query blocks per strip

F32 = mybir.dt.float32
BF16 = mybir.dt.bfloat16


def _emit(ctx, tc, xT, wq, wk, wv, wo, bq, bk, bv, masks, out):
    nc = tc.nc
    EXP = mybir.ActivationFunctionType.Exp

    const = ctx.enter_context(tc.tile_pool(name="const", bufs=1))
    p_pt = ctx.enter_context(tc.tile_pool(name="p_pt", bufs=4))
    p_small = ctx.enter_context(tc.tile_pool(name="p_small", bufs=2))
    p_dram = ctx.enter_context(tc.tile_pool(name="p_dram", bufs=2, space="DRAM"))
    # PSUM budget (8 banks): p_st 2x2 + p_pv 1x2 + p_aux 2x1 = 8.
    p_st = ctx.enter_context(tc.tile_pool(name="p_st", bufs=2, space="PSUM"))
    p_pv = ctx.enter_context(tc.tile_pool(name="p_pv", bufs=1, space="PSUM"))
    p_aux = ctx.enter_context(tc.tile_pool(name="p_aux", bufs=2, space="PSUM"))
    p_osb = ctx.enter_context(tc.tile_pool(name="p_osb", bufs=3))

    # constants on the GpSimd (SWDGE) queue so they don't serialize with the
    # x^T stream on the sync (HWDGE) queue. maskt = one triangular 0/1 tile.
    maskt = const.tile([P, P], BF16)
    nc.gpsimd.dma_start(out=maskt, in_=masks)
    bqt = const.tile([P, NSTRIP], F32)
    nc.gpsimd.dma_start(out=bqt, in_=bq.rearrange("(s p) -> p s", p=P))
    bkt = const.tile([P, NSTRIP], F32)
    nc.gpsimd.dma_start(out=bkt, in_=bk.rearrange("(s p) -> p s", p=P))
    bvb = const.tile([P, DC], F32)      # loaded after Wv on this queue

    # persistent per-batch tensors
    attnT = const.tile([P, NSTRIP, N], BF16)                # normalized attn^T
    vplus = const.tile([P, NTT, HPC, HD + 1], BF16)         # V | ones column
    # memset on a bf16 matmul-input tile is invalid ISA; write the ones
    # column via a DVE copy from an f32 staging tile (a valid rounding producer)
    ones_f32 = const.tile([P, NTT * HPC], F32)
    nc.vector.memset(ones_f32, 1.0)
    nc.vector.tensor_copy(
        out=vplus[:, :, :, HD:HD + 1],
        in_=ones_f32.rearrange("p (a b) -> p a b", b=HPC).unsqueeze(3),
    )
    # selectors for the PE-broadcast normalize: [128,128] stationaries with
    # one all-ones row (32*qs) and zeros elsewhere, so lhsT.T @ recip_rows
    # replicates that reciprocal row across all 128 PSUM partitions. The
    # reciprocal input stays full-width at partition 0 (custom-DVE ops
    # misbehave on partition-offset slices); unselected moving rows are
    # finite (sums tiles are memset 1.0), so the 0-weights stay exact.
    sel_f32 = const.tile([P, P], F32)
    nc.vector.memset(sel_f32, 0.0)
    sel_bf = []
    for i in range(NTS):
        sb = const.tile([P, P], BF16, name=f"sel{i}")
        nc.vector.tensor_copy(out=sb, in_=sel_f32)
        nc.vector.tensor_copy(
            out=sb[32 * i:32 * i + 1, :],
            in_=ones_f32[0:1, 0:P])
        sel_bf.append(sb)

    # Wo tile loaded after strip-0 weights (not needed until phase C); single
    # multi-chunk descriptor per tensor -- each dma_start costs ~600ns of issue
    # time on its queue engine, so batching chunks into one AP matters. All
    # weight/x DRAM tensors arrive host-pre-tiled so every DMA reads 8KB
    # contiguous per partition (1KB segments halve effective DMA bandwidth).
    wot = const.tile([P, NSTRIP, D], BF16)

    with tc.tile_pool(name="p_xt", bufs=1) as p_xt:
        # x^T resident, 64KB/part total, one tile per strip so dependency
        # tracking can't couple strip-0 consumers to strip 1-3 DMAs.
        # Strip 0 is split into three tiles (c0 | c1-3 | c4-7) so the first
        # QK matmul starts after only one weight chunk + one x chunk land.
        xts = [p_xt.tile([P, NDIN // 2, QW], BF16, name="xts0a"),
               p_xt.tile([P, NDIN // 2, QW], BF16, name="xts0b")]
        xts += [p_xt.tile([P, NDIN, QW], BF16, name=f"xts{t}")
                for t in range(1, NTS)]

        def xtile(t, c):
            if t == 0:
                return xts[c // 4][:, c % 4, :]
            return xts[t + 1][:, c, :]

        with (
            tc.tile_pool(name="p_w", bufs=2) as p_w,
            tc.tile_pool(name="p_wv", bufs=1) as p_wv,
            tc.tile_pool(name="p_qk", bufs=2) as p_qk,
        ):
            # the sync queue moves ~4x the bytes/ns of the scalar/gpsimd
            # queues, so the whole critical sequence rides it in exact
            # consumption order. Only deferrable loads (bv, strip>=1
            # weights, Wo) go elsewhere.
            wqs0 = p_w.tile([P, NDIN, P], BF16, tag="wq")
            wks0 = p_w.tile([P, NDIN, P], BF16, tag="wk")
            nc.sync.dma_start(out=wqs0, in_=wq[0])
            nc.sync.dma_start(out=xts[0], in_=xT[:, 0, 0:4])
            nc.sync.dma_start(out=wks0, in_=wk[0])
            nc.sync.dma_start(out=xts[1], in_=xT[:, 0, 4:8])
            nc.sync.dma_start(out=xts[2], in_=xT[:, 1])
            nc.sync.dma_start(out=xts[3], in_=xT[:, 2])
            nc.sync.dma_start(out=xts[4], in_=xT[:, 3])
            # Wv rides the gpsimd (SWDGE) queue: slower per-byte, but it
            # runs in parallel with the x^T stream on the sync queue, so
            # V-proj starts earlier AND the x strips land sooner
            wvt = p_wv.tile([P, NDIN, DC], BF16)
            nc.gpsimd.dma_start(out=wvt, in_=wv)
            nc.gpsimd.dma_start(
                out=bvb, in_=bv.unsqueeze(0).partition_broadcast(P))

            st = {}
            pending = [None]

            def make_strip(s):
                if s == 0:
                    def wq_at(c):
                        return wqs0[:, c, :]

                    def wk_at(c):
                        return wks0[:, c, :]
                else:
                    wqs = p_w.tile([P, NDIN, P], BF16, tag="wq")
                    wks = p_w.tile([P, NDIN, P], BF16, tag="wk")
                    nc.gpsimd.dma_start(out=wqs, in_=wq[s])
                    nc.gpsimd.dma_start(out=wks, in_=wk[s])

                    def wq_at(c, w=wqs):
                        return w[:, c, :]

                    def wk_at(c, w=wks):
                        return w[:, c, :]
                if s == 1:
                    nc.gpsimd.dma_start(out=wot, in_=wo)
                qts = p_qk.tile([P, N], BF16, tag="qt")
                kts = p_qk.tile([P, N], BF16, tag="kt")
                # sums rows at partition offsets {0,32,64,96} (DVE partition
                # offsets must be 32-aligned); unused rows are memset to 1.0
                # so the batched reciprocal stays finite
                sums_sb = p_small.tile([P, 2, QW], F32, tag="sums")
                nc.gpsimd.memset(sums_sb, 1.0)
                st[s] = (wq_at, wk_at, qts, kts, sums_sb)

            def s_units(qs):
                """Unit list for one query strip: full-width key blocks
                below the diagonal region, then three diagonal units with
                shrinking query widths -- queries before the key block are
                skipped entirely, the remaining 128-wide leading wedge of
                each unit gets the triangular mask."""
                nfull = NQB * qs
                us = [("full", kb) for kb in range(nfull)]
                us += [("diag0", nfull), ("diag1", nfull), ("diagB", nfull)]
                return us

            def emit_s(s, qs, kind, kb):
                """S^T + exp for both heads of one unit. The two heads'
                S matmuls (contraction 64, partition bases 0/64) occupy
                disjoint PE row-tiles and run concurrently; one activation
                exps both heads' scores."""
                _, _, qts, kts, _ = st[s]
                q0 = qs * QW
                pst = p_st.tile([P, 2, QW], F32, tag="st", name="pst")
                pt = p_pt.tile([P, 2, QW], BF16, tag="pt", name="pt")
                if kind == "full" or kind == "diag0":
                    for h2 in range(2):
                        po = h2 * HD
                        nc.tensor.matmul(
                            pst[:, h2, :],
                            lhsT=kts[po:po + HD, ts(kb, P)],
                            rhs=qts[po:po + HD, ts(qs, QW)],
                            start=True, stop=True,
                        )
                    nc.scalar.activation(out=pt, in_=pst, func=EXP,
                                         scale=0.125)
                    if kind == "diag0":
                        for h2 in range(2):
                            nc.vector.tensor_mul(
                                pt[:, h2, 0:P], pt[:, h2, 0:P], maskt)
                    return pt
                if kind == "diag1":
                    # kc = kb+1, queries [128:512), tri on cols 0:128
                    for h2 in range(2):
                        po = h2 * HD
                        nc.tensor.matmul(
                            pst[:, h2, 0:3 * P],
                            lhsT=kts[po:po + HD, ts(kb + 1, P)],
                            rhs=qts[po:po + HD, ds(q0 + P, 3 * P)],
                            start=True, stop=True,
                        )
                    nc.scalar.activation(
                        out=pt[:, :, 0:3 * P], in_=pst[:, :, 0:3 * P],
                        func=EXP, scale=0.125)
                    for h2 in range(2):
                        nc.vector.tensor_mul(
                            pt[:, h2, 0:P], pt[:, h2, 0:P], maskt)
                    return pt
                # diagB: kc=kb+2, queries [256:512) at cols 0:256;
                #        kc=kb+3, queries [384:512) at cols 256:384
                for h2 in range(2):
                    po = h2 * HD
                    nc.tensor.matmul(
                        pst[:, h2, 0:2 * P],
                        lhsT=kts[po:po + HD, ts(kb + 2, P)],
                        rhs=qts[po:po + HD, ds(q0 + 2 * P, 2 * P)],
                        start=True, stop=True,
                    )
                for h2 in range(2):
                    po = h2 * HD
                    nc.tensor.matmul(
                        pst[:, h2, 2 * P:3 * P],
                        lhsT=kts[po:po + HD, ts(kb + 3, P)],
                        rhs=qts[po:po + HD, ds(q0 + 3 * P, P)],
                        start=True, stop=True,
                    )
                nc.scalar.activation(
                    out=pt[:, :, 0:3 * P], in_=pst[:, :, 0:3 * P],
                    func=EXP, scale=0.125)
                for h2 in range(2):
                    nc.vector.tensor_mul(
                        pt[:, h2, 0:P], pt[:, h2, 0:P], maskt)
                    nc.vector.tensor_mul(
                        pt[:, h2, 2 * P:3 * P], pt[:, h2, 2 * P:3 * P],
                        maskt)
                return pt

            def emit_pv(s, kind, kb, pt, pvp):
                for h2 in range(2):
                    h = 2 * s + h2
                    if kind == "full" or kind == "diag0":
                        nc.tensor.matmul(
                            pvp[:, h2, :], lhsT=vplus[:, kb, h, :],
                            rhs=pt[:, h2, :],
                            start=(kb == 0), stop=False,
                        )
                    elif kind == "diag1":
                        nc.tensor.matmul(
                            pvp[:, h2, P:4 * P],
                            lhsT=vplus[:, kb + 1, h, :],
                            rhs=pt[:, h2, 0:3 * P],
                            start=False, stop=False,
                        )
                    else:
                        nc.tensor.matmul(
                            pvp[:, h2, 2 * P:4 * P],
                            lhsT=vplus[:, kb + 2, h, :],
                            rhs=pt[:, h2, 0:2 * P],
                            start=False, stop=False,
                        )
                        nc.tensor.matmul(
                            pvp[:, h2, 3 * P:4 * P],
                            lhsT=vplus[:, kb + 3, h, :],
                            rhs=pt[:, h2, 2 * P:3 * P],
                            start=False, stop=True,
                        )

            def evict_qs(s, qs, pvp):
                # evict both heads in parallel (scalar takes one attnT copy,
                # vector the other): pvp is single-buffered, so these gate
                # the next query strip's first PV matmuls
                _, _, _, _, sums_sb = st[s]
                for h2 in range(2):
                    nc.vector.tensor_copy(
                        out=sums_sb[32 * qs:32 * qs + 1, h2, :],
                        in_=pvp[HD:HD + 1, h2, :])
                nc.scalar.copy(
                    out=attnT[0:HD, s, ts(qs, QW)], in_=pvp[0:HD, 0, :])
                nc.vector.tensor_copy(
                    out=attnT[HD:2 * HD, s, ts(qs, QW)], in_=pvp[0:HD, 1, :])

            def flat_attention(items, filler=None, drain_hook=None):
                """LOOKP-pipelined S/exp stream with PV trailing, flat
                across query-strip and head-strip boundaries so the PE
                never drains between them. items: (s, qs, kind, kb, last).
                drain_hook(s, qs) runs after each strip's eviction."""
                LOOKP = 2
                pts = {}
                pvps = {}
                n = len(items)
                # units advance in pairs: two S-pairs (64,128 PE tiling
                # mode) back-to-back, then two PV groups (128,128 mode),
                # halving the tiling-mode switches vs per-unit alternation
                for base in range(0, n + LOOKP, 2):
                    for i in (base, base + 1):
                        if i < n:
                            s, qs, kind, kb, _ = items[i]
                            pts[i] = emit_s(s, qs, kind, kb)
                        if filler is not None:
                            next(filler, None)
                    for i in (base, base + 1):
                        j = i - LOOKP
                        if not (0 <= j < n):
                            continue
                        s, qs, kind, kb, last = items[j]
                        if (s, qs) not in pvps:
                            pvps[(s, qs)] = p_pv.tile(
                                [HD + 1, 2, QW], F32, tag="pv", name="pvp")
                        emit_pv(s, kind, kb, pts.pop(j), pvps[(s, qs)])
                        if last:
                            evict_qs(s, qs, pvps.pop((s, qs)))
                            if drain_hook is not None:
                                drain_hook(s, qs)

            def attn_pair(s, qs, filler=None):
                us = s_units(qs)
                items = [(s, qs, kind, kb, i == len(us) - 1)
                         for i, (kind, kb) in enumerate(us)]
                flat_attention(items, filler)

            def normalize_h2(s, h2, sums_sb):
                """Batched softmax normalization for one head (4 query strips).

                1/s via the fast custom-DVE reciprocal; normalize multiplies
                run on GpSimd to keep Vector free for the inner-loop copies.
                """
                po = h2 * HD
                recip_sb = p_small.tile([P, QW], F32, tag="recip",
                                        name="recip_sb")
                nc.vector.reciprocal_approx_fast(
                    out=recip_sb, in_=sums_sb[:, h2, :])
                recb_sb = p_small.tile([P, QW], BF16, tag="recb",
                                       name="recb_sb")
                nc.vector.tensor_copy(out=recb_sb, in_=recip_sb)
                # broadcast across partitions via a DRAM round-trip
                # (SBUF-source partition-broadcast DMA is rejected); bf16
                # halves the 1MB/head broadcast traffic
                recip_d = p_dram.tile([NTS, QW], BF16, tag="recipd",
                                      name="recip_d")
                nc.sync.dma_start(
                    out=recip_d,
                    in_=recb_sb.rearrange("(a b) f -> a b f", b=32)[:, 0, :])
                # full-128-partition broadcast so rb[po:po+HD] shares the
                # base partition with the attnT slice (DVE rule); all 4
                # query strips in one issue
                rb = p_small.tile([P, NTS, QW], BF16, tag="rb", bufs=2,
                                  name="rb")
                nc.sync.dma_start(
                    out=rb, in_=recip_d.unsqueeze(0).partition_broadcast(P))
                for qs in range(NTS):
                    sl = attnT[po:po + HD, s, ts(qs, QW)]
                    nc.gpsimd.tensor_mul(
                        out=sl, in0=sl, in1=rb[po:po + HD, qs, :])

            def normalize_bcast_qs(s, h2, qs, sums_sb):
                """Per-query-strip normalize for the LAST strip: reciprocal
                of the sums rows, with the wanted row replicated across
                partitions by a one-hot PE matmul (sel[128,128] x
                recip[128,512] -> PSUM[128,512]), then one DVE multiply.
                ~2us end-to-end vs ~6us for the DRAM round-trip broadcast,
                so the interleaved O-projection isn't gated on a long
                store/load chain."""
                po = h2 * HD
                recq = p_small.tile([P, QW], F32, tag="recq", name="recq")
                nc.vector.reciprocal_approx_fast(
                    out=recq, in_=sums_sb[:, h2, :])
                recb = p_small.tile([P, QW], BF16, tag="recb2", name="recb2")
                # the bf16 cast rides scalar for one head so the two heads'
                # chains run on different engines
                if h2:
                    nc.scalar.copy(out=recb, in_=recq)
                else:
                    nc.vector.tensor_copy(out=recb, in_=recq)
                rbq = p_aux.tile([P, QW], F32, tag="mm", name="rbq")
                nc.tensor.matmul(
                    rbq, lhsT=sel_bf[qs], rhs=recb,
                    start=True, stop=True)
                sl = attnT[po:po + HD, s, ts(qs, QW)]
                nc.vector.tensor_mul(
                    out=sl, in0=sl, in1=rbq[po:po + HD, :])

            def emit_qk0(t):
                """Strip-0 Q/K projection for one token strip (plain order:
                runs against the incoming x/weight DMA stream)."""
                wq_at, wk_at, qts, kts, _ = st[0]
                psq = p_aux.tile([P, QW], F32, tag="mm", name="psq")
                for c in range(NDIN):
                    nc.tensor.matmul(
                        psq, lhsT=wq_at(c), rhs=xtile(t, c),
                        start=(c == 0), stop=(c == NDIN - 1),
                    )
                nc.vector.tensor_scalar_add(
                    out=qts[:, ts(t, QW)], in0=psq, scalar1=bqt[:, 0:1])
                psk = p_aux.tile([P, QW], F32, tag="mm", name="psk")
                for c in range(NDIN):
                    nc.tensor.matmul(
                        psk, lhsT=wk_at(c), rhs=xtile(t, c),
                        start=(c == 0), stop=(c == NDIN - 1),
                    )
                nc.vector.tensor_scalar_add(
                    out=kts[:, ts(t, QW)], in0=psk, scalar1=bkt[:, 0:1])

            def emit_v(t):
                # V = x @ Wv + bv, one token strip at a time right
                # before the attention group that first needs it
                for tt in range(NQB * t, NQB * (t + 1)):
                    psv = p_aux.tile([P, DC], F32, tag="mm", name="psv")
                    for c in range(NDIN):
                        nc.tensor.matmul(
                            psv,
                            lhsT=xtile(t, c)[:, ts(tt % NQB, P)],
                            rhs=wvt[:, c, :],
                            start=(c == 0), stop=(c == NDIN - 1),
                        )
                    nc.vector.tensor_add(
                        out=vplus[:, tt, :, 0:HD],
                        in0=psv.rearrange("p (h d) -> p h d", d=HD),
                        in1=bvb.rearrange("p (h d) -> p h d", d=HD),
                    )

            def qk_pair_gen(s, t0s):
                """Chunk-major Q/K projection for strip s over the given
                token-strip pairs: each weight chunk is loaded once as the
                PE stationary and used for two token strips' matmuls.
                Yields after every chunk (~2 matmuls) so the attention loop
                can pull fine-grained PE filler."""
                wq_at, wk_at, qts, kts, _ = st[s]
                for w_at, dst, bias in ((wq_at, qts, bqt), (wk_at, kts, bkt)):
                    for t0 in t0s:
                        ps0 = p_aux.tile([P, QW], F32, tag="mm", name="ps0")
                        ps1 = p_aux.tile([P, QW], F32, tag="mm", name="ps1")
                        for c in range(NDIN):
                            nc.tensor.matmul(
                                ps0, lhsT=w_at(c), rhs=xtile(t0, c),
                                start=(c == 0), stop=(c == NDIN - 1))
                            nc.tensor.matmul(
                                ps1, lhsT=w_at(c), rhs=xtile(t0 + 1, c),
                                start=(c == 0), stop=(c == NDIN - 1))
                            yield
                        nc.vector.tensor_scalar_add(
                            out=dst[:, ts(t0, QW)], in0=ps0,
                            scalar1=bias[:, s:s + 1])
                        nc.vector.tensor_scalar_add(
                            out=dst[:, ts(t0 + 1, QW)], in0=ps1,
                            scalar1=bias[:, s:s + 1])
                        yield

            def qk_part1(s):
                """Strip s's setup + Q/K for tokens 0:1024 -- pulled as PE
                filler during strip s-1's attention. Tokens 1024:2048
                (qk_part2) are deferred into strip s's own first two query
                strips, which otherwise have no filler work."""
                make_strip(s)
                yield
                yield from qk_pair_gen(s, (0,))

            def qk_part2(s):
                yield from qk_pair_gen(s, (2,))

            def phase_c_tts(tts):
                """O-projection for the given token tiles: partial output =
                attnT^T @ Wo_slice. The stationary attnT chunk is shared by
                the two output-half matmuls."""
                for tt in tts:
                    pso0 = p_aux.tile([P, QW], F32, tag="mm", name="pso0")
                    pso1 = p_aux.tile([P, QW], F32, tag="mm", name="pso1")
                    for c in range(NSTRIP):
                        nc.tensor.matmul(
                            pso0, lhsT=attnT[:, c, ts(tt, P)],
                            rhs=wot[:, c, ds(0, QW)],
                            start=(c == 0), stop=(c == NSTRIP - 1))
                        nc.tensor.matmul(
                            pso1, lhsT=attnT[:, c, ts(tt, P)],
                            rhs=wot[:, c, ds(QW, QW)],
                            start=(c == 0), stop=(c == NSTRIP - 1))
                    # bf16 out + store each half immediately: halves the
                    # store bytes and drains right after the copy
                    osb = p_osb.tile([P, D], BF16, tag="osb", name="osb")
                    nc.vector.tensor_copy(out=osb[:, ds(0, QW)], in_=pso0)
                    nc.sync.dma_start(
                        out=out[ts(tt, P), ds(0, QW)], in_=osb[:, ds(0, QW)])
                    nc.vector.tensor_copy(out=osb[:, ds(QW, QW)], in_=pso1)
                    nc.sync.dma_start(
                        out=out[ts(tt, P), ds(QW, QW)],
                        in_=osb[:, ds(QW, QW)])

            # ---- strip 0: software-pipelined against the DMA stream ----
            make_strip(0)
            _, _, _, _, sums0 = st[0]
            emit_qk0(0)
            qk1 = None
            for t in range(NTS):
                if t + 1 < NTS:
                    emit_qk0(t + 1)
                emit_v(t)
                attn_pair(0, t)
                if t == 0:
                    qk1 = qk_part1(1)
                for _ in range(9):
                    next(qk1, None)
            for _ in qk1:
                pass
            pending[0] = (lambda sb=sums0:
                          (normalize_h2(0, 0, sb),
                           normalize_h2(0, 1, sb)))

            # ---- strips 1-3 ----
            for s in range(1, NSTRIP):
                _, _, _, _, sums_sb = st[s]
                last = (s == NSTRIP - 1)
                filler = itertools.chain(
                    qk_part2(s),
                    qk_part1(s + 1) if not last else iter(()))
                for qs in range(NTS):
                    if last and qs >= 1:
                        # one token tile held back from the previous group
                        # goes FIRST: it is ready-to-run PE work covering
                        # this normalize chain's latency (critical for the
                        # final one, where no attention work remains)
                        if qs >= 2:
                            phase_c_tts([NQB * (qs - 2) + 3])
                        normalize_bcast_qs(s, 0, qs - 1, sums_sb)
                        normalize_bcast_qs(s, 1, qs - 1, sums_sb)
                        phase_c_tts(range(NQB * (qs - 1), NQB * qs - 1))
                    attn_pair(s, qs, filler)
                    if pending[0] is not None and qs == (0 if last else 1):
                        pending[0]()
                        pending[0] = None
                if not last:
                    for _ in filler:
                        pass
                    pending[0] = (lambda ss=s, sb=sums_sb:
                                  (normalize_h2(ss, 0, sb),
                                   normalize_h2(ss, 1, sb)))
                else:
                    phase_c_tts([NQB * (NTS - 2) + 3])
                    normalize_bcast_qs(s, 0, NTS - 1, sums_sb)
                    normalize_bcast_qs(s, 1, NTS - 1, sums_sb)
                    phase_c_tts(range(NQB * (NTS - 1), NTT))


_emit_wrapped = with_exitstack(_emit)

_NC_CACHE = None


def _build():
    global _NC_CACHE
    if _NC_CACHE is not None:
        return _NC_CACHE
    nc = bacc.Bacc("TRN2", target_bir_lowering=False, debug=False)
    # all inputs host-pre-tiled to the SBUF tile layouts (contiguous
    # per-partition runs -> minimal DMA descriptors)
    xT = nc.dram_tensor(
        "xt", [P, NTS, NDIN, QW], BF16, kind="ExternalInput").ap()
    wq = nc.dram_tensor(
        "wq", [NSTRIP, P, NDIN, P], BF16, kind="ExternalInput").ap()
    wk = nc.dram_tensor(
        "wk", [NSTRIP, P, NDIN, P], BF16, kind="ExternalInput").ap()
    wv = nc.dram_tensor(
        "wv", [P, NDIN, DC], BF16, kind="ExternalInput").ap()
    wo = nc.dram_tensor(
        "wo", [P, NSTRIP, D], BF16, kind="ExternalInput").ap()
    bq = nc.dram_tensor("bq", [DC], F32, kind="ExternalInput").ap()
    bk = nc.dram_tensor("bk", [DC], F32, kind="ExternalInput").ap()
    bv = nc.dram_tensor("bv", [DC], F32, kind="ExternalInput").ap()
    masks = nc.dram_tensor("masks", [P, P], BF16, kind="ExternalInput").ap()
    out = nc.dram_tensor("out", [N, D], BF16, kind="ExternalOutput").ap()
    with tile.TileContext(nc) as tc:
        _emit_wrapped(tc, xT, wq, wk, wv, wo, bq, bk, bv, masks, out)
    nc.compile()
    _NC_CACHE = nc
    return nc


def _make_masks():
    # triangular 0/1 tile for the diagonal blocks of S^T: key <= query kept
    return np.triu(np.ones((P, P), np.float32)).astype(ml_dtypes.bfloat16)


def _in_maps(x, Wq, bq, Wk, bk, Wv, bv, Wo):
    masks = _make_masks()
    bf = ml_dtypes.bfloat16
    maps = []
    # pre-tile into the exact SBUF layouts (see _build dram shapes)
    wq_g, wk_g, wv_g, wo_g = [], [], [], []
    for g in range(GROUPS):
        sl = slice(g * DC, (g + 1) * DC)
        wq_g.append(np.ascontiguousarray(
            Wq[:, sl].reshape(NDIN, P, NSTRIP, P).transpose(2, 1, 0, 3)
        ).astype(bf))
        wk_g.append(np.ascontiguousarray(
            Wk[:, sl].reshape(NDIN, P, NSTRIP, P).transpose(2, 1, 0, 3)
        ).astype(bf))
        wv_g.append(np.ascontiguousarray(
            Wv[:, sl].reshape(NDIN, P, DC).transpose(1, 0, 2)).astype(bf))
        wo_g.append(np.ascontiguousarray(
            Wo[sl, :].reshape(NSTRIP, P, D).transpose(1, 0, 2)).astype(bf))
    for b in range(B):
        xt_b = np.ascontiguousarray(
            np.asarray(x[b]).T.reshape(NDIN, P, NTS, QW).transpose(1, 2, 0, 3)
        ).astype(bf)
        for g in range(GROUPS):
            sl = slice(g * DC, (g + 1) * DC)
            maps.append({
                "xt": xt_b,
                "wq": wq_g[g],
                "wk": wk_g[g],
                "wv": wv_g[g],
                "wo": wo_g[g],
                "bq": np.ascontiguousarray(bq[sl]),
                "bk": np.ascontiguousarray(bk[sl]),
                "bv": np.ascontiguousarray(bv[sl]),
                "masks": masks,
            })
    return maps


def run(inputs, trace=False, tmpdir=None):
    """Build+run on 8 cores. Returns (out [B,N,D] f32, BassKernelResults)."""
    x = np.asarray(inputs["x"], np.float32)
    args = [np.asarray(inputs[k], np.float32) for k in
            ("Wq", "bq", "Wk", "bk", "Wv", "bv", "Wo")]
    bo = np.asarray(inputs["bo"], np.float32)
    nc = _build()
    maps = _in_maps(x, *args)
    if trace:
        bass_utils.upload_artifacts = lambda d: d
    res = bass_utils.run_bass_kernel_spmd(
        nc, maps, core_ids=list(range(8)), trace=trace, tmpdir=tmpdir)
    out = np.empty((B, N, D), np.float32)
    for b in range(B):
        out[b] = (res.results[2 * b]["out"].astype(np.float32)
                  + res.results[2 * b + 1]["out"].astype(np.float32) + bo)
    return out, res


def kernel(**inputs):
    out, _ = run(inputs)
    return out


# revision 43
# speedup vs baseline: 1.1879x; 1.0082x over previous
"""Multi-head causal self-attention (B=4, N=2048, D=1024, H=16) on 8 TRN2 cores.

Sharding: 8 cores = 4 batches x 2 head-groups (8 heads / 512 dims each).
Per core (batch b, group g):
  - QKV projections computed in transposed layout (dims on partitions):
      Q^T, K^T = W^T-chunks (lhsT) x x^T (rhs), accumulated over 8 din chunks.
      V computed in natural [token, dv] layout (lhsT = x^T chunk).
  - Attention computed as S^T tiles [keys(128) x queries(512)] so that
    exp(S) feeds the P^T.V matmul directly (contraction over keys on
    partitions, no transposes anywhere). The two heads of a 128-dim strip
    occupy SBUF partitions 0:64 / 64:128, so their S^T matmuls (contraction
    64 = head_dim) lower to PE row-tiles (64,0)/(64,128-mode T0/T8) and run
    CONCURRENTLY in the array -- emitted back-to-back into one 2-bank PSUM
    tile, exp'd in a single wide activation covering both heads.
    Softmax denominators come from a ones-column appended to V (row HD of
    the PV accumulator); normalization is deferred and batched per strip.
    Causal masking = skip blocks above the diagonal + multiply
    diagonal-region tiles by a precomputed 0/1 mask after exp. No
    max-subtraction: scores are ~N(0,1) after the 1/sqrt(hd) scale.
  - O-projection partial: attnT (lhsT) x Wo-slice (rhs) -> [2048, 1024]
    partial output per core; host sums the two group partials per batch.
    O-proj for early token tiles is interleaved into the last strip's
    attention (per-query-strip fast normalize) so the PE never idles
    waiting on the scalar engine's exp throughput.

Dtypes: bf16 matmuls everywhere (full PE rate); f32 PSUM accumulation.
"""

import itertools

import numpy as np
import ml_dtypes

import concourse.bass as bass
import concourse.tile as tile
from concourse import bacc, mybir
from concourse import bass_utils
from concourse._compat import with_exitstack
from concourse.bass import ts, ds

B, N, D, H, HD = 4, 2048, 1024, 16, 64
GROUPS = 2              # head groups (cores per batch)
DC = D // GROUPS        # 512 dims per core
HPC = H // GROUPS       # 8 heads per core
P = 128
QW = 512                # query strip width / matmul free dim
NDIN = D // P           # 8 contraction chunks for QKV
NSTRIP = DC // P        # 4 dq strips per core (2 heads each)
NTT = N // P            # 16 token tiles
NTS = N // QW           # 4 token strips
NQB = QW // P           # 4 query blocks per strip

F32 = mybir.dt.float32
BF16 = mybir.dt.bfloat16


def _emit(ctx, tc, xT, wq, wk, wv, wo, bq, bk, bv, masks, out):
    nc = tc.nc
    EXP = mybir.ActivationFunctionType.Exp

    const = ctx.enter_context(tc.tile_pool(name="const", bufs=1))
    p_pt = ctx.enter_context(tc.tile_pool(name="p_pt", bufs=4))
    p_small = ctx.enter_context(tc.tile_pool(name="p_small", bufs=2))
    p_dram = ctx.enter_context(tc.tile_pool(name="p_dram", bufs=2, space="DRAM"))
    # PSUM budget (8 banks): p_st 2x2 + p_pv 1x2 + p_aux 2x1 = 8.
    p_st = ctx.enter_context(tc.tile_pool(name="p_st", bufs=2, space="PSUM"))
    p_pv = ctx.enter_context(tc.tile_pool(name="p_pv", bufs=1, space="PSUM"))
    p_aux = ctx.enter_context(tc.tile_pool(name="p_aux", bufs=2, space="PSUM"))
    p_osb = ctx.enter_context(tc.tile_pool(name="p_osb", bufs=3))

    # constants on the GpSimd (SWDGE) queue so they don't serialize with the
    # x^T stream on the sync (HWDGE) queue. maskt = one triangular 0/1 tile.
    maskt = const.tile([P, P], BF16)
    nc.gpsimd.dma_start(out=maskt, in_=masks)
    bqt = const.tile([P, NSTRIP], F32)
    nc.gpsimd.dma_start(out=bqt, in_=bq.rearrange("(s p) -> p s", p=P))
    bkt = const.tile([P, NSTRIP], F32)
    nc.gpsimd.dma_start(out=bkt, in_=bk.rearrange("(s p) -> p s", p=P))
    bvb = const.tile([P, DC], F32)      # loaded after strip-0 criticals

    # persistent per-batch tensors
    attnT = const.tile([P, NSTRIP, N], BF16)                # normalized attn^T
    vplus = const.tile([P, NTT, HPC, HD + 1], BF16)         # V | ones column
    # memset on a bf16 matmul-input tile is invalid ISA; write the ones
    # column via a DVE copy from an f32 staging tile (a valid rounding producer)
    ones_f32 = const.tile([P, NTT * HPC], F32)
    nc.vector.memset(ones_f32, 1.0)
    nc.vector.tensor_copy(
        out=vplus[:, :, :, HD:HD + 1],
        in_=ones_f32.rearrange("p (a b) -> p a b", b=HPC).unsqueeze(3),
    )
    # selectors for the PE-broadcast normalize: [128,128] stationaries with
    # one all-ones row (32*qs) and zeros elsewhere, so lhsT.T @ recip_rows
    # replicates that reciprocal row across all 128 PSUM partitions. The
    # reciprocal input stays full-width at partition 0 (custom-DVE ops
    # misbehave on partition-offset slices); unselected moving rows are
    # finite (sums tiles are memset 1.0), so the 0-weights stay exact.
    sel_f32 = const.tile([P, P], F32)
    nc.vector.memset(sel_f32, 0.0)
    sel_bf = []
    for i in range(NTS):
        sb = const.tile([P, P], BF16, name=f"sel{i}")
        nc.vector.tensor_copy(out=sb, in_=sel_f32)
        nc.vector.tensor_copy(
            out=sb[32 * i:32 * i + 1, :],
            in_=ones_f32[0:1, 0:P])
        sel_bf.append(sb)

    # Wo tile loaded after strip-0 weights (not needed until phase C); single
    # multi-chunk descriptor per tensor -- each dma_start costs ~600ns of issue
    # time on its queue engine, so batching chunks into one AP matters. All
    # weight/x DRAM tensors arrive host-pre-tiled so every DMA reads 8KB
    # contiguous per partition (1KB segments halve effective DMA bandwidth).
    wot = const.tile([P, NSTRIP, D], BF16)

    with tc.tile_pool(name="p_xt", bufs=1) as p_xt:
        # x^T resident, 64KB/part total, one tile per strip so dependency
        # tracking can't couple strip-0 consumers to strip 1-3 DMAs.
        # Strip 0 is split into three tiles (c0 | c1-3 | c4-7) so the first
        # QK matmul starts after only one weight chunk + one x chunk land.
        xts = [p_xt.tile([P, NDIN // 2, QW], BF16, name="xts0a"),
               p_xt.tile([P, NDIN // 2, QW], BF16, name="xts0b")]
        xts += [p_xt.tile([P, NDIN, QW], BF16, name=f"xts{t}")
                for t in range(1, NTS)]

        def xtile(t, c):
            if t == 0:
                return xts[c // 4][:, c % 4, :]
            return xts[t + 1][:, c, :]

        with (
            tc.tile_pool(name="p_w", bufs=2) as p_w,
            tc.tile_pool(name="p_wv", bufs=1) as p_wv,
            tc.tile_pool(name="p_qk", bufs=2) as p_qk,
        ):
            # the sync queue moves ~4x the bytes/ns of the scalar/gpsimd
            # queues, so the whole critical sequence rides it in exact
            # consumption order. Only deferrable loads (bv, strip>=1
            # weights, Wo) go elsewhere.
            wqs0 = p_w.tile([P, NDIN, P], BF16, tag="wq")
            wks0 = p_w.tile([P, NDIN, P], BF16, tag="wk")
            nc.sync.dma_start(out=wqs0, in_=wq[0])
            nc.sync.dma_start(out=xts[0], in_=xT[:, 0, 0:4])
            nc.sync.dma_start(out=wks0, in_=wk[0])
            nc.sync.dma_start(out=xts[1], in_=xT[:, 0, 4:8])
            nc.sync.dma_start(out=xts[2], in_=xT[:, 1])
            wvt = p_wv.tile([P, NDIN, DC], BF16)
            nc.sync.dma_start(out=wvt, in_=wv)
            nc.sync.dma_start(out=xts[3], in_=xT[:, 2])
            nc.sync.dma_start(out=xts[4], in_=xT[:, 3])
            nc.gpsimd.dma_start(
                out=bvb, in_=bv.unsqueeze(0).partition_broadcast(P))

            st = {}
            pending = [None]

            def make_strip(s):
                if s == 0:
                    def wq_at(c):
                        return wqs0[:, c, :]

                    def wk_at(c):
                        return wks0[:, c, :]
                else:
                    wqs = p_w.tile([P, NDIN, P], BF16, tag="wq")
                    wks = p_w.tile([P, NDIN, P], BF16, tag="wk")
                    nc.gpsimd.dma_start(out=wqs, in_=wq[s])
                    nc.gpsimd.dma_start(out=wks, in_=wk[s])

                    def wq_at(c, w=wqs):
                        return w[:, c, :]

                    def wk_at(c, w=wks):
                        return w[:, c, :]
                if s == 1:
                    nc.gpsimd.dma_start(out=wot, in_=wo)
                qts = p_qk.tile([P, N], BF16, tag="qt")
                kts = p_qk.tile([P, N], BF16, tag="kt")
                # sums rows at partition offsets {0,32,64,96} (DVE partition
                # offsets must be 32-aligned); unused rows are memset to 1.0
                # so the batched reciprocal stays finite
                sums_sb = p_small.tile([P, 2, QW], F32, tag="sums")
                nc.gpsimd.memset(sums_sb, 1.0)
                st[s] = (wq_at, wk_at, qts, kts, sums_sb)

            def s_units(qs):
                """Unit list for one query strip: full-width key blocks
                below the diagonal region, then three diagonal units with
                shrinking query widths -- queries before the key block are
                skipped entirely, the remaining 128-wide leading wedge of
                each unit gets the triangular mask."""
                nfull = NQB * qs
                us = [("full", kb) for kb in range(nfull)]
                us += [("diag0", nfull), ("diag1", nfull), ("diagB", nfull)]
                return us

            def emit_s(s, qs, kind, kb):
                """S^T + exp for both heads of one unit. The two heads'
                S matmuls (contraction 64, partition bases 0/64) occupy
                disjoint PE row-tiles and run concurrently; one activation
                exps both heads' scores."""
                _, _, qts, kts, _ = st[s]
                q0 = qs * QW
                pst = p_st.tile([P, 2, QW], F32, tag="st", name="pst")
                pt = p_pt.tile([P, 2, QW], BF16, tag="pt", name="pt")
                if kind == "full" or kind == "diag0":
                    for h2 in range(2):
                        po = h2 * HD
                        nc.tensor.matmul(
                            pst[:, h2, :],
                            lhsT=kts[po:po + HD, ts(kb, P)],
                            rhs=qts[po:po + HD, ts(qs, QW)],
                            start=True, stop=True,
                        )
                    nc.scalar.activation(out=pt, in_=pst, func=EXP,
                                         scale=0.125)
                    if kind == "diag0":
                        for h2 in range(2):
                            nc.vector.tensor_mul(
                                pt[:, h2, 0:P], pt[:, h2, 0:P], maskt)
                    return pt
                if kind == "diag1":
                    # kc = kb+1, queries [128:512), tri on cols 0:128
                    for h2 in range(2):
                        po = h2 * HD
                        nc.tensor.matmul(
                            pst[:, h2, 0:3 * P],
                            lhsT=kts[po:po + HD, ts(kb + 1, P)],
                            rhs=qts[po:po + HD, ds(q0 + P, 3 * P)],
                            start=True, stop=True,
                        )
                    nc.scalar.activation(
                        out=pt[:, :, 0:3 * P], in_=pst[:, :, 0:3 * P],
                        func=EXP, scale=0.125)
                    for h2 in range(2):
                        nc.vector.tensor_mul(
                            pt[:, h2, 0:P], pt[:, h2, 0:P], maskt)
                    return pt
                # diagB: kc=kb+2, queries [256:512) at cols 0:256;
                #        kc=kb+3, queries [384:512) at cols 256:384
                for h2 in range(2):
                    po = h2 * HD
                    nc.tensor.matmul(
                        pst[:, h2, 0:2 * P],
                        lhsT=kts[po:po + HD, ts(kb + 2, P)],
                        rhs=qts[po:po + HD, ds(q0 + 2 * P, 2 * P)],
                        start=True, stop=True,
                    )
                for h2 in range(2):
                    po = h2 * HD
                    nc.tensor.matmul(
                        pst[:, h2, 2 * P:3 * P],
                        lhsT=kts[po:po + HD, ts(kb + 3, P)],
                        rhs=qts[po:po + HD, ds(q0 + 3 * P, P)],
                        start=True, stop=True,
                    )
                nc.scalar.activation(
                    out=pt[:, :, 0:3 * P], in_=pst[:, :, 0:3 * P],
                    func=EXP, scale=0.125)
                for h2 in range(2):
                    nc.vector.tensor_mul(
                        pt[:, h2, 0:P], pt[:, h2, 0:P], maskt)
                    nc.vector.tensor_mul(
                        pt[:, h2, 2 * P:3 * P], pt[:, h2, 2 * P:3 * P],
                        maskt)
                return pt

            def emit_pv(s, kind, kb, pt, pvp):
                for h2 in range(2):
                    h = 2 * s + h2
                    if kind == "full" or kind == "diag0":
                        nc.tensor.matmul(
                            pvp[:, h2, :], lhsT=vplus[:, kb, h, :],
                            rhs=pt[:, h2, :],
                            start=(kb == 0), stop=False,
                        )
                    elif kind == "diag1":
                        nc.tensor.matmul(
                            pvp[:, h2, P:4 * P],
                            lhsT=vplus[:, kb + 1, h, :],
                            rhs=pt[:, h2, 0:3 * P],
                            start=False, stop=False,
                        )
                    else:
                        nc.tensor.matmul(
                            pvp[:, h2, 2 * P:4 * P],
                            lhsT=vplus[:, kb + 2, h, :],
                            rhs=pt[:, h2, 0:2 * P],
                            start=False, stop=False,
                        )
                        nc.tensor.matmul(
                            pvp[:, h2, 3 * P:4 * P],
                            lhsT=vplus[:, kb + 3, h, :],
                            rhs=pt[:, h2, 2 * P:3 * P],
                            start=False, stop=True,
                        )

            def evict_qs(s, qs, pvp):
                # evict both heads in parallel (scalar takes one attnT copy,
                # vector the other): pvp is single-buffered, so these gate
                # the next query strip's first PV matmuls
                _, _, _, _, sums_sb = st[s]
                for h2 in range(2):
                    nc.vector.tensor_copy(
                        out=sums_sb[32 * qs:32 * qs + 1, h2, :],
                        in_=pvp[HD:HD + 1, h2, :])
                nc.scalar.copy(
                    out=attnT[0:HD, s, ts(qs, QW)], in_=pvp[0:HD, 0, :])
                nc.vector.tensor_copy(
                    out=attnT[HD:2 * HD, s, ts(qs, QW)], in_=pvp[0:HD, 1, :])

            def flat_attention(items, filler=None, drain_hook=None):
                """LOOKP-pipelined S/exp stream with PV trailing, flat
                across query-strip and head-strip boundaries so the PE
                never drains between them. items: (s, qs, kind, kb, last).
                drain_hook(s, qs) runs after each strip's eviction."""
                LOOKP = 2
                pts = {}
                pvps = {}
                n = len(items)
                for i in range(n + LOOKP):
                    if i < n:
                        s, qs, kind, kb, _ = items[i]
                        pts[i] = emit_s(s, qs, kind, kb)
                    if filler is not None:
                        next(filler, None)
                    if i >= LOOKP:
                        s, qs, kind, kb, last = items[i - LOOKP]
                        if (s, qs) not in pvps:
                            pvps[(s, qs)] = p_pv.tile(
                                [HD + 1, 2, QW], F32, tag="pv", name="pvp")
                        emit_pv(s, kind, kb, pts.pop(i - LOOKP),
                                pvps[(s, qs)])
                        if last:
                            evict_qs(s, qs, pvps.pop((s, qs)))
                            if drain_hook is not None:
                                drain_hook(s, qs)

            def attn_pair(s, qs, filler=None):
                us = s_units(qs)
                items = [(s, qs, kind, kb, i == len(us) - 1)
                         for i, (kind, kb) in enumerate(us)]
                flat_attention(items, filler)

            def normalize_h2(s, h2, sums_sb):
                """Batched softmax normalization for one head (4 query strips).

                1/s via the fast custom-DVE reciprocal; normalize multiplies
                run on GpSimd to keep Vector free for the inner-loop copies.
                """
                po = h2 * HD
                recip_sb = p_small.tile([P, QW], F32, tag="recip",
                                        name="recip_sb")
                nc.vector.reciprocal_approx_fast(
                    out=recip_sb, in_=sums_sb[:, h2, :])
                recb_sb = p_small.tile([P, QW], BF16, tag="recb",
                                       name="recb_sb")
                nc.vector.tensor_copy(out=recb_sb, in_=recip_sb)
                # broadcast across partitions via a DRAM round-trip
                # (SBUF-source partition-broadcast DMA is rejected); bf16
                # halves the 1MB/head broadcast traffic
                recip_d = p_dram.tile([NTS, QW], BF16, tag="recipd",
                                      name="recip_d")
                nc.sync.dma_start(
                    out=recip_d,
                    in_=recb_sb.rearrange("(a b) f -> a b f", b=32)[:, 0, :])
                # full-128-partition broadcast so rb[po:po+HD] shares the
                # base partition with the attnT slice (DVE rule); all 4
                # query strips in one issue
                rb = p_small.tile([P, NTS, QW], BF16, tag="rb", bufs=2,
                                  name="rb")
                nc.sync.dma_start(
                    out=rb, in_=recip_d.unsqueeze(0).partition_broadcast(P))
                for qs in range(NTS):
                    sl = attnT[po:po + HD, s, ts(qs, QW)]
                    nc.gpsimd.tensor_mul(
                        out=sl, in0=sl, in1=rb[po:po + HD, qs, :])

            def normalize_bcast_qs(s, h2, qs, sums_sb):
                """Per-query-strip normalize for the LAST strip: reciprocal
                of the sums rows, with the wanted row replicated across
                partitions by a one-hot PE matmul (sel[128,128] x
                recip[128,512] -> PSUM[128,512]), then one DVE multiply.
                ~2us end-to-end vs ~6us for the DRAM round-trip broadcast,
                so the interleaved O-projection isn't gated on a long
                store/load chain."""
                po = h2 * HD
                recq = p_small.tile([P, QW], F32, tag="recq", name="recq")
                nc.vector.reciprocal_approx_fast(
                    out=recq, in_=sums_sb[:, h2, :])
                recb = p_small.tile([P, QW], BF16, tag="recb2", name="recb2")
                # the bf16 cast rides scalar for one head so the two heads'
                # chains run on different engines
                if h2:
                    nc.scalar.copy(out=recb, in_=recq)
                else:
                    nc.vector.tensor_copy(out=recb, in_=recq)
                rbq = p_aux.tile([P, QW], F32, tag="mm", name="rbq")
                nc.tensor.matmul(
                    rbq, lhsT=sel_bf[qs], rhs=recb,
                    start=True, stop=True)
                sl = attnT[po:po + HD, s, ts(qs, QW)]
                nc.vector.tensor_mul(
                    out=sl, in0=sl, in1=rbq[po:po + HD, :])

            def emit_qk0(t):
                """Strip-0 Q/K projection for one token strip (plain order:
                runs against the incoming x/weight DMA stream)."""
                wq_at, wk_at, qts, kts, _ = st[0]
                psq = p_aux.tile([P, QW], F32, tag="mm", name="psq")
                for c in range(NDIN):
                    nc.tensor.matmul(
                        psq, lhsT=wq_at(c), rhs=xtile(t, c),
                        start=(c == 0), stop=(c == NDIN - 1),
                    )
                nc.vector.tensor_scalar_add(
                    out=qts[:, ts(t, QW)], in0=psq, scalar1=bqt[:, 0:1])
                psk = p_aux.tile([P, QW], F32, tag="mm", name="psk")
                for c in range(NDIN):
                    nc.tensor.matmul(
                        psk, lhsT=wk_at(c), rhs=xtile(t, c),
                        start=(c == 0), stop=(c == NDIN - 1),
                    )
                nc.vector.tensor_scalar_add(
                    out=kts[:, ts(t, QW)], in0=psk, scalar1=bkt[:, 0:1])

            def emit_v(t):
                # V = x @ Wv + bv, one token strip at a time right
                # before the attention group that first needs it
                for tt in range(NQB * t, NQB * (t + 1)):
                    psv = p_aux.tile([P, DC], F32, tag="mm", name="psv")
                    for c in range(NDIN):
                        nc.tensor.matmul(
                            psv,
                            lhsT=xtile(t, c)[:, ts(tt % NQB, P)],
                            rhs=wvt[:, c, :],
                            start=(c == 0), stop=(c == NDIN - 1),
                        )
                    nc.vector.tensor_add(
                        out=vplus[:, tt, :, 0:HD],
                        in0=psv.rearrange("p (h d) -> p h d", d=HD),
                        in1=bvb.rearrange("p (h d) -> p h d", d=HD),
                    )

            def qk_pair_gen(s, t0s):
                """Chunk-major Q/K projection for strip s over the given
                token-strip pairs: each weight chunk is loaded once as the
                PE stationary and used for two token strips' matmuls.
                Yields after every chunk (~2 matmuls) so the attention loop
                can pull fine-grained PE filler."""
                wq_at, wk_at, qts, kts, _ = st[s]
                for w_at, dst, bias in ((wq_at, qts, bqt), (wk_at, kts, bkt)):
                    for t0 in t0s:
                        ps0 = p_aux.tile([P, QW], F32, tag="mm", name="ps0")
                        ps1 = p_aux.tile([P, QW], F32, tag="mm", name="ps1")
                        for c in range(NDIN):
                            nc.tensor.matmul(
                                ps0, lhsT=w_at(c), rhs=xtile(t0, c),
                                start=(c == 0), stop=(c == NDIN - 1))
                            nc.tensor.matmul(
                                ps1, lhsT=w_at(c), rhs=xtile(t0 + 1, c),
                                start=(c == 0), stop=(c == NDIN - 1))
                            yield
                        nc.vector.tensor_scalar_add(
                            out=dst[:, ts(t0, QW)], in0=ps0,
                            scalar1=bias[:, s:s + 1])
                        nc.vector.tensor_scalar_add(
                            out=dst[:, ts(t0 + 1, QW)], in0=ps1,
                            scalar1=bias[:, s:s + 1])
                        yield

            def qk_part1(s):
                """Strip s's setup + Q/K for tokens 0:1024 -- pulled as PE
                filler during strip s-1's attention. Tokens 1024:2048
                (qk_part2) are deferred into strip s's own first two query
                strips, which otherwise have no filler work."""
                make_strip(s)
                yield
                yield from qk_pair_gen(s, (0,))

            def qk_part2(s):
                yield from qk_pair_gen(s, (2,))

            def phase_c_tts(tts):
                """O-projection for the given token tiles: partial output =
                attnT^T @ Wo_slice. The stationary attnT chunk is shared by
                the two output-half matmuls."""
                for tt in tts:
                    pso0 = p_aux.tile([P, QW], F32, tag="mm", name="pso0")
                    pso1 = p_aux.tile([P, QW], F32, tag="mm", name="pso1")
                    for c in range(NSTRIP):
                        nc.tensor.matmul(
                            pso0, lhsT=attnT[:, c, ts(tt, P)],
                            rhs=wot[:, c, ds(0, QW)],
                            start=(c == 0), stop=(c == NSTRIP - 1))
                        nc.tensor.matmul(
                            pso1, lhsT=attnT[:, c, ts(tt, P)],
                            rhs=wot[:, c, ds(QW, QW)],
                            start=(c == 0), stop=(c == NSTRIP - 1))
                    # bf16 out + store each half immediately: halves the
                    # store bytes and drains right after the copy
                    osb = p_osb.tile([P, D], BF16, tag="osb", name="osb")
                    nc.vector.tensor_copy(out=osb[:, ds(0, QW)], in_=pso0)
                    nc.sync.dma_start(
                        out=out[ts(tt, P), ds(0, QW)], in_=osb[:, ds(0, QW)])
                    nc.vector.tensor_copy(out=osb[:, ds(QW, QW)], in_=pso1)
                    nc.sync.dma_start(
                        out=out[ts(tt, P), ds(QW, QW)],
                        in_=osb[:, ds(QW, QW)])

            # ---- strip 0: software-pipelined against the DMA stream ----
            make_strip(0)
            _, _, _, _, sums0 = st[0]
            emit_qk0(0)
            qk1 = None
            for t in range(NTS):
                if t + 1 < NTS:
                    emit_qk0(t + 1)
                emit_v(t)
                attn_pair(0, t)
                if t == 0:
                    qk1 = qk_part1(1)
                for _ in range(9):
                    next(qk1, None)
            for _ in qk1:
                pass
            pending[0] = (lambda sb=sums0:
                          (normalize_h2(0, 0, sb),
                           normalize_h2(0, 1, sb)))

            # ---- strips 1-3 ----
            for s in range(1, NSTRIP):
                _, _, _, _, sums_sb = st[s]
                last = (s == NSTRIP - 1)
                filler = itertools.chain(
                    qk_part2(s),
                    qk_part1(s + 1) if not last else iter(()))
                for qs in range(NTS):
                    if last and qs >= 1:
                        # one token tile held back from the previous group
                        # goes FIRST: it is ready-to-run PE work covering
                        # this normalize chain's latency (critical for the
                        # final one, where no attention work remains)
                        if qs >= 2:
                            phase_c_tts([NQB * (qs - 2) + 3])
                        normalize_bcast_qs(s, 0, qs - 1, sums_sb)
                        normalize_bcast_qs(s, 1, qs - 1, sums_sb)
                        phase_c_tts(range(NQB * (qs - 1), NQB * qs - 1))
                    attn_pair(s, qs, filler)
                    if pending[0] is not None and qs == (0 if last else 1):
                        pending[0]()
                        pending[0] = None
                if not last:
                    for _ in filler:
                        pass
                    pending[0] = (lambda ss=s, sb=sums_sb:
                                  (normalize_h2(ss, 0, sb),
                                   normalize_h2(ss, 1, sb)))
                else:
                    phase_c_tts([NQB * (NTS - 2) + 3])
                    normalize_bcast_qs(s, 0, NTS - 1, sums_sb)
                    normalize_bcast_qs(s, 1, NTS - 1, sums_sb)
                    phase_c_tts(range(NQB * (NTS - 1), NTT))


_emit_wrapped = with_exitstack(_emit)

_NC_CACHE = None


def _build():
    global _NC_CACHE
    if _NC_CACHE is not None:
        return _NC_CACHE
    nc = bacc.Bacc("TRN2", target_bir_lowering=False, debug=False)
    # all inputs host-pre-tiled to the SBUF tile layouts (contiguous
    # per-partition runs -> minimal DMA descriptors)
    xT = nc.dram_tensor(
        "xt", [P, NTS, NDIN, QW], BF16, kind="ExternalInput").ap()
    wq = nc.dram_tensor(
        "wq", [NSTRIP, P, NDIN, P], BF16, kind="ExternalInput").ap()
    wk = nc.dram_tensor(
        "wk", [NSTRIP, P, NDIN, P], BF16, kind="ExternalInput").ap()
    wv = nc.dram_tensor(
        "wv", [P, NDIN, DC], BF16, kind="ExternalInput").ap()
    wo = nc.dram_tensor(
        "wo", [P, NSTRIP, D], BF16, kind="ExternalInput").ap()
    bq = nc.dram_tensor("bq", [DC], F32, kind="ExternalInput").ap()
    bk = nc.dram_tensor("bk", [DC], F32, kind="ExternalInput").ap()
    bv = nc.dram_tensor("bv", [DC], F32, kind="ExternalInput").ap()
    masks = nc.dram_tensor("masks", [P, P], BF16, kind="ExternalInput").ap()
    out = nc.dram_tensor("out", [N, D], BF16, kind="ExternalOutput").ap()
    with tile.TileContext(nc) as tc:
        _emit_wrapped(tc, xT, wq, wk, wv, wo, bq, bk, bv, masks, out)
    nc.compile()
    _NC_CACHE = nc
    return nc


def _make_masks():
    # triangular 0/1 tile for the diagonal blocks of S^T: key <= query kept
    return np.triu(np.ones((P, P), np.float32)).astype(ml_dtypes.bfloat16)


def _in_maps(x, Wq, bq, Wk, bk, Wv, bv, Wo):
    masks = _make_masks()
    bf = ml_dtypes.bfloat16
    maps = []
    # pre-tile into the exact SBUF layouts (see _build dram shapes)
    wq_g, wk_g, wv_g, wo_g = [], [], [], []
    for g in range(GROUPS):
        sl = slice(g * DC, (g + 1) * DC)
        wq_g.append(np.ascontiguousarray(
            Wq[:, sl].reshape(NDIN, P, NSTRIP, P).transpose(2, 1, 0, 3)
        ).astype(bf))
        wk_g.append(np.ascontiguousarray(
            Wk[:, sl].reshape(NDIN, P, NSTRIP, P).transpose(2, 1, 0, 3)
        ).astype(bf))
        wv_g.append(np.ascontiguousarray(
            Wv[:, sl].reshape(NDIN, P, DC).transpose(1, 0, 2)).astype(bf))
        wo_g.append(np.ascontiguousarray(
            Wo[sl, :].reshape(NSTRIP, P, D).transpose(1, 0, 2)).astype(bf))
    for b in range(B):
        xt_b = np.ascontiguousarray(
            np.asarray(x[b]).T.reshape(NDIN, P, NTS, QW).transpose(1, 2, 0, 3)
        ).astype(bf)
        for g in range(GROUPS):
            sl = slice(g * DC, (g + 1) * DC)
            maps.append({
                "xt": xt_b,
                "wq": wq_g[g],
                "wk": wk_g[g],
                "wv": wv_g[g],
                "wo": wo_g[g],
                "bq": np.ascontiguousarray(bq[sl]),
                "bk": np.ascontiguousarray(bk[sl]),
                "bv": np.ascontiguousarray(bv[sl]),
                "masks": masks,
            })
    return maps


def run(inputs, trace=False, tmpdir=None):
    """Build+run on 8 cores. Returns (out [B,N,D] f32, BassKernelResults)."""
    x = np.asarray(inputs["x"], np.float32)
    args = [np.asarray(inputs[k], np.float32) for k in
            ("Wq", "bq", "Wk", "bk", "Wv", "bv", "Wo")]
    bo = np.asarray(inputs["bo"], np.float32)
    nc = _build()
    maps = _in_maps(x, *args)
    if trace:
        bass_utils.upload_artifacts = lambda d: d
    res = bass_utils.run_bass_kernel_spmd(
        nc, maps, core_ids=list(range(8)), trace=trace, tmpdir=tmpdir)
    out = np.empty((B, N, D), np.float32)
    for b in range(B):
        out[b] = (res.results[2 * b]["out"].astype(np.float32)
                  + res.results[2 * b + 1]["out"].astype(np.float32) + bo)
    return out, res


def kernel(**inputs):
    out, _ = run(inputs)
    return out
